# revision 19
# baseline (speedup 1.0000x reference)
"""Modulated 1x1 conv (ModConv) on 8 Trainium2 NeuronCores.

out[b,o,h,w] = sum_c (style[b,c] * weight[o,c]) * x[b,c,h,w]

Strategy: pure data parallel over the batch — 2 samples per core. Per
sample the kernel modulates the (pre-transposed) weight with the style
vector on DVE (cheap: [512,128] elements), then runs a K=512 contraction
as 4 PSUM-accumulated matmuls per 512-wide output tile. Matmul operands
use float32r (TF32-like PE path: full-rate rows vs 4 cycles/row for
fp32, ~1e-4 rel err), so the problem is HBM-bound (~21 MB/core at
~360 GB/s). x streams in as 1 MB [128, 2048] chunks alternating between
the SP and ACT HWDGE rings; outputs leave via the gpsimd SWDGE ring so
they never stall the input streams.
"""

import numpy as np

import concourse.bass as bass
import concourse.mybir as mybir
from concourse.bass_utils import run_bass_kernel_spmd
from concourse.tile import TileContext

B, CIN, COUT, H, W = 16, 512, 128, 64, 64
HW = H * W
N_CORES = 8
BPC = B // N_CORES  # samples per core
P = 128
KT = CIN // P  # k-tiles per contraction
NTILE = 512  # one PSUM bank of fp32
NT = HW // NTILE
NHALF = 2  # x chunks per k-tile (n-direction)
NCHUNK = HW // NHALF
FP32 = mybir.dt.float32
F32R = mybir.dt.float32r

# This container's walrus (public-SDK build) accepts at most one sync
# wait command per instruction; Tile's sem assignment attaches one wait
# per depended-on proc. Hoist the excess onto dedicated wait
# instructions (the same InstEventSemaphore a bass `wait_ge` emits)
# immediately before the over-subscribed instruction on its own engine.
MAX_WAITS_PER_INST = 1


def _split_sync_waits(nc: bass.Bass, limit: int = MAX_WAITS_PER_INST) -> int:
    n_split = 0
    for f in nc.m.functions:
        for bb in f.blocks:
            out = []
            for ins in bb.instructions:
                si = getattr(ins, "sync_info", None)
                if si is not None and si.on_wait and len(si.on_wait) > limit:
                    waits = list(si.on_wait)
                    for w in waits[:-limit]:
                        n_split += 1
                        es = mybir.InstEventSemaphore(
                            name=f"{ins.name}-ws{n_split}",
                            opcode="EventSemaphore",
                            engine=ins.engine,
                            sync_info=mybir.SyncInfo(on_wait=[w], on_update=[]),
                        )
                        nc.register_instruction(es, overwrite=True)
                        out.append(es)
                    si.on_wait = waits[-limit:]
                out.append(ins)
            bb.instructions[:] = out
    return n_split


def build_kernel(
    reps: int = 1,
    bench_mode: bool = False,
    nhalf: int = NHALF,
    x_bufs: int | None = None,
    psum_bufs: int = 4,
    skip_out: bool = False,
    skip_compute: bool = False,
    out_every: int | None = None,
    x_three_queues: bool = False,
) -> bass.Bass:
    """reps>1 replicates the whole per-sample pipeline in-program (same
    inputs, outputs rewritten) — used only by the bench to measure
    steady-state per-iteration time with per-call overhead cancelled.
    bench_mode writes the big output to internal DRAM and exposes only a
    4-byte token output, so per-call tunnel traffic is negligible."""
    nchunk = HW // nhalf
    if x_bufs is None:
        x_bufs = 2 * KT * nhalf
    if out_every is None:
        out_every = NT // nhalf
    nc = bass.Bass()
    x = nc.dram_tensor("x", [BPC, CIN, HW], F32R, kind="ExternalInput")
    styleT = nc.dram_tensor("styleT", [CIN, BPC], FP32, kind="ExternalInput")
    wT = nc.dram_tensor("wT", [CIN, COUT], FP32, kind="ExternalInput")
    if bench_mode:
        out = nc.dram_tensor("out_scratch", [BPC, COUT, HW], FP32)
        token = nc.dram_tensor("token", [1, 1], FP32, kind="ExternalOutput")
    else:
        out = nc.dram_tensor("out", [BPC, COUT, HW], FP32, kind="ExternalOutput")
        token = None

    # The two HWDGE rings (SP + ACT) stream x in parallel.
    x_dma_engines = [nc.sync, nc.scalar]
    if x_three_queues:
        x_dma_engines = [nc.sync, nc.scalar, nc.gpsimd]

    with TileContext(nc) as tc:
        with (
            tc.tile_pool(name="consts", bufs=1) as cpool,
            tc.tile_pool(name="xs", bufs=x_bufs) as xpool,
            tc.tile_pool(name="os", bufs=2) as opool,
            tc.tile_pool(name="ps", bufs=psum_bufs, space="PSUM") as pspool,
        ):
            wT_sb = cpool.tile([P, KT, COUT], FP32)
            nc.sync.dma_start(out=wT_sb[:], in_=wT[:].rearrange("(t p) o -> p t o", p=P))
            sT_sb = cpool.tile([P, KT, BPC], FP32)
            nc.scalar.dma_start(
                out=sT_sb[:], in_=styleT[:].rearrange("(t p) b -> p t b", p=P)
            )
            # Per-sample modulated (transposed) weights: mw[p, b, t, o].
            # Stored as float32r so the PE takes the fast fp32 path.
            mw_sb = cpool.tile([P, BPC, KT, COUT], F32R)
            for b in range(BPC):
                for t in range(KT):
                    nc.vector.tensor_scalar_mul(
                        mw_sb[:, b, t, :], wT_sb[:, t, :], sT_sb[:, t, b : b + 1]
                    )

            dma_i = 0
            for _rep in range(reps):
                for b in range(BPC):
                    # x chunks: xh[t][h] = x[b, t*128:(t+1)*128, h*nchunk:(h+1)*nchunk]
                    xh = [[None] * nhalf for _ in range(KT)]
                    for h in range(nhalf):
                        for t in range(KT):
                            xt = xpool.tile([P, nchunk], F32R, tag="xt")
                            eng = x_dma_engines[dma_i % len(x_dma_engines)]
                            dma_i += 1
                            eng.dma_start(
                                out=xt[:],
                                in_=x[
                                    b,
                                    t * P : (t + 1) * P,
                                    h * nchunk : (h + 1) * nchunk,
                                ],
                            )
                            xh[t][h] = xt
                    if skip_compute:
                        continue
                    ot = opool.tile([P, HW], FP32, tag="ot")
                    for n in range(NT):
                        h, j = divmod(n, NT // nhalf)
                        ps = pspool.tile([P, NTILE], FP32, tag="ps")
                        for t in range(KT):
                            nc.tensor.matmul(
                                ps[:],
                                mw_sb[:, b, t, :],
                                xh[t][h][:, j * NTILE : (j + 1) * NTILE],
                                start=(t == 0),
                                stop=(t == KT - 1),
                            )
                        nc.vector.tensor_copy(
                            out=ot[:, n * NTILE : (n + 1) * NTILE], in_=ps[:]
                        )
                        if not skip_out and (n + 1) % out_every == 0:
                            lo = (n + 1 - out_every) * NTILE
                            hi = (n + 1) * NTILE
                            nc.gpsimd.dma_start(
                                out=out[b, :, lo:hi], in_=ot[:, lo:hi]
                            )
            if token is not None:
                nc.gpsimd.dma_start(out=token[:], in_=mw_sb[:1, 0, 0, :1])

    _split_sync_waits(nc)
    return nc


_NC_CACHE: bass.Bass | None = None


def _get_nc() -> bass.Bass:
    global _NC_CACHE
    if _NC_CACHE is None:
        _NC_CACHE = build_kernel()
    return _NC_CACHE


def make_in_maps(x: np.ndarray, style: np.ndarray, weight: np.ndarray):
    x_flat = np.ascontiguousarray(np.asarray(x, dtype=np.float32)).reshape(B, CIN, HW)
    styleT = np.ascontiguousarray(np.asarray(style, dtype=np.float32).T)  # [CIN, B]
    wT = np.ascontiguousarray(np.asarray(weight, dtype=np.float32).T)  # [CIN, COUT]
    in_maps = []
    for c in range(N_CORES):
        sl = slice(c * BPC, (c + 1) * BPC)
        in_maps.append(
            {
                "x": x_flat[sl],
                "styleT": np.ascontiguousarray(styleT[:, sl]),
                "wT": wT,
            }
        )
    return in_maps


def gather_out(results) -> np.ndarray:
    out = np.empty((B, COUT, H, W), dtype=np.float32)
    for c in range(N_CORES):
        out[c * BPC : (c + 1) * BPC] = results[c]["out"].reshape(BPC, COUT, H, W)
    return out


def kernel(x: np.ndarray, style: np.ndarray, weight: np.ndarray) -> np.ndarray:
    nc = _get_nc()
    in_maps = make_in_maps(x, style, weight)
    res = run_bass_kernel_spmd(nc, in_maps, core_ids=list(range(N_CORES)))
    return gather_out(res.results)


# revision 21
# speedup vs baseline: 45369.7831x; 45369.7831x over previous
"""Modulated 1x1 conv (ModConv) on 8 Trainium2 NeuronCores.

out[b,o,h,w] = sum_c (style[b,c] * weight[o,c]) * x[b,c,h,w]

Strategy: pure data parallel over the batch — 2 samples per core. Per
sample the kernel modulates the (pre-transposed) weight with the style
vector on DVE (cheap: [512,128] elements), then runs a K=512 contraction
as 4 PSUM-accumulated matmuls per 512-wide output tile. Matmul operands
use float32r (TF32-like PE path: full-rate rows vs 4 cycles/row for
fp32, ~1e-4 rel err), so the problem is HBM-bound (~21 MB/core at
~360 GB/s). x streams in as 1 MB [128, 2048] chunks alternating between
the SP and ACT HWDGE rings; outputs leave via the gpsimd SWDGE ring so
they never stall the input streams.
"""

import numpy as np

import concourse.bass as bass
import concourse.mybir as mybir
from concourse.bass_utils import run_bass_kernel_spmd
from concourse.tile import TileContext

B, CIN, COUT, H, W = 16, 512, 128, 64, 64
HW = H * W
N_CORES = 8
BPC = B // N_CORES  # samples per core
P = 128
KT = CIN // P  # k-tiles per contraction
NTILE = 512  # one PSUM bank of fp32
NT = HW // NTILE
NHALF = 2  # x chunks per k-tile (n-direction)
NCHUNK = HW // NHALF
FP32 = mybir.dt.float32
F32R = mybir.dt.float32r

# This container's walrus (public-SDK build) accepts at most one sync
# wait command per instruction; Tile's sem assignment attaches one wait
# per depended-on proc. Hoist the excess onto dedicated wait
# instructions (the same InstEventSemaphore a bass `wait_ge` emits)
# immediately before the over-subscribed instruction on its own engine.
MAX_WAITS_PER_INST = 1


def _split_sync_waits(nc: bass.Bass, limit: int = MAX_WAITS_PER_INST) -> int:
    n_split = 0
    for f in nc.m.functions:
        for bb in f.blocks:
            out = []
            for ins in bb.instructions:
                si = getattr(ins, "sync_info", None)
                if si is not None and si.on_wait and len(si.on_wait) > limit:
                    waits = list(si.on_wait)
                    for w in waits[:-limit]:
                        n_split += 1
                        es = mybir.InstEventSemaphore(
                            name=f"{ins.name}-ws{n_split}",
                            opcode="EventSemaphore",
                            engine=ins.engine,
                            sync_info=mybir.SyncInfo(on_wait=[w], on_update=[]),
                        )
                        nc.register_instruction(es, overwrite=True)
                        out.append(es)
                    si.on_wait = waits[-limit:]
                out.append(ins)
            bb.instructions[:] = out
    return n_split


def build_kernel(
    reps: int = 1,
    bench_mode: bool = False,
    nhalf: int = NHALF,
    x_bufs: int | None = None,
    psum_bufs: int = 4,
    skip_out: bool = False,
    skip_compute: bool = False,
    out_every: int | None = None,
    x_three_queues: bool = False,
    o_bufs: int = 2,
) -> bass.Bass:
    """reps>1 replicates the whole per-sample pipeline in-program (same
    inputs, outputs rewritten) — used only by the bench to measure
    steady-state per-iteration time with per-call overhead cancelled.
    bench_mode writes the big output to internal DRAM and exposes only a
    4-byte token output, so per-call tunnel traffic is negligible."""
    nchunk = HW // nhalf
    if x_bufs is None:
        x_bufs = 2 * KT * nhalf
    if out_every is None:
        out_every = NT // nhalf
    nc = bass.Bass()
    x = nc.dram_tensor("x", [BPC, CIN, HW], F32R, kind="ExternalInput")
    styleT = nc.dram_tensor("styleT", [CIN, BPC], FP32, kind="ExternalInput")
    wT = nc.dram_tensor("wT", [CIN, COUT], FP32, kind="ExternalInput")
    if bench_mode:
        out = nc.dram_tensor("out_scratch", [BPC, COUT, HW], FP32)
        token = nc.dram_tensor("token", [1, 1], FP32, kind="ExternalOutput")
    else:
        out = nc.dram_tensor("out", [BPC, COUT, HW], FP32, kind="ExternalOutput")
        token = None

    # The two HWDGE rings (SP + ACT) stream x in parallel.
    x_dma_engines = [nc.sync, nc.scalar]
    if x_three_queues:
        x_dma_engines = [nc.sync, nc.scalar, nc.gpsimd]

    with TileContext(nc) as tc:
        with (
            tc.tile_pool(name="consts", bufs=1) as cpool,
            tc.tile_pool(name="xs", bufs=x_bufs) as xpool,
            tc.tile_pool(name="os", bufs=o_bufs) as opool,
            tc.tile_pool(name="ps", bufs=psum_bufs, space="PSUM") as pspool,
        ):
            wT_sb = cpool.tile([P, KT, COUT], FP32)
            nc.sync.dma_start(out=wT_sb[:], in_=wT[:].rearrange("(t p) o -> p t o", p=P))
            sT_sb = cpool.tile([P, KT, BPC], FP32)
            nc.scalar.dma_start(
                out=sT_sb[:], in_=styleT[:].rearrange("(t p) b -> p t b", p=P)
            )
            # Per-sample modulated (transposed) weights: mw[p, b, t, o].
            # Stored as float32r so the PE takes the fast fp32 path.
            mw_sb = cpool.tile([P, BPC, KT, COUT], F32R)
            for b in range(BPC):
                for t in range(KT):
                    nc.vector.tensor_scalar_mul(
                        mw_sb[:, b, t, :], wT_sb[:, t, :], sT_sb[:, t, b : b + 1]
                    )

            dma_i = 0
            for _rep in range(reps):
                for b in range(BPC):
                    # x chunks: xh[t][h] = x[b, t*128:(t+1)*128, h*nchunk:(h+1)*nchunk]
                    xh = [[None] * nhalf for _ in range(KT)]
                    for h in range(nhalf):
                        for t in range(KT):
                            xt = xpool.tile([P, nchunk], F32R, tag="xt")
                            eng = x_dma_engines[dma_i % len(x_dma_engines)]
                            dma_i += 1
                            eng.dma_start(
                                out=xt[:],
                                in_=x[
                                    b,
                                    t * P : (t + 1) * P,
                                    h * nchunk : (h + 1) * nchunk,
                                ],
                            )
                            xh[t][h] = xt
                    if skip_compute:
                        continue
                    ot = opool.tile([P, HW], FP32, tag="ot")
                    for n in range(NT):
                        h, j = divmod(n, NT // nhalf)
                        ps = pspool.tile([P, NTILE], FP32, tag="ps")
                        for t in range(KT):
                            nc.tensor.matmul(
                                ps[:],
                                mw_sb[:, b, t, :],
                                xh[t][h][:, j * NTILE : (j + 1) * NTILE],
                                start=(t == 0),
                                stop=(t == KT - 1),
                            )
                        nc.vector.tensor_copy(
                            out=ot[:, n * NTILE : (n + 1) * NTILE], in_=ps[:]
                        )
                        if not skip_out and (n + 1) % out_every == 0:
                            lo = (n + 1 - out_every) * NTILE
                            hi = (n + 1) * NTILE
                            nc.gpsimd.dma_start(
                                out=out[b, :, lo:hi], in_=ot[:, lo:hi]
                            )
            if token is not None:
                nc.gpsimd.dma_start(out=token[:], in_=mw_sb[:1, 0, 0, :1])

    _split_sync_waits(nc)
    return nc


_NC_CACHE: bass.Bass | None = None


def _get_nc() -> bass.Bass:
    global _NC_CACHE
    if _NC_CACHE is None:
        _NC_CACHE = build_kernel()
    return _NC_CACHE


def make_in_maps(x: np.ndarray, style: np.ndarray, weight: np.ndarray):
    x_flat = np.ascontiguousarray(np.asarray(x, dtype=np.float32)).reshape(B, CIN, HW)
    styleT = np.ascontiguousarray(np.asarray(style, dtype=np.float32).T)  # [CIN, B]
    wT = np.ascontiguousarray(np.asarray(weight, dtype=np.float32).T)  # [CIN, COUT]
    in_maps = []
    for c in range(N_CORES):
        sl = slice(c * BPC, (c + 1) * BPC)
        in_maps.append(
            {
                "x": x_flat[sl],
                "styleT": np.ascontiguousarray(styleT[:, sl]),
                "wT": wT,
            }
        )
    return in_maps


def gather_out(results) -> np.ndarray:
    out = np.empty((B, COUT, H, W), dtype=np.float32)
    for c in range(N_CORES):
        out[c * BPC : (c + 1) * BPC] = results[c]["out"].reshape(BPC, COUT, H, W)
    return out


def kernel(x: np.ndarray, style: np.ndarray, weight: np.ndarray) -> np.ndarray:
    nc = _get_nc()
    in_maps = make_in_maps(x, style, weight)
    res = run_bass_kernel_spmd(nc, in_maps, core_ids=list(range(N_CORES)))
    return gather_out(res.results)
